# revision 18
# baseline (speedup 1.0000x reference)
"""Conv2d 3x3 same-padding, NCHW, on 8 TRN2 NeuronCores (data-parallel).

Problem: x[32,128,56,56] f32, weight[256,128,3,3] OIHW, bias[256] ->
y[32,256,56,56].  Batch is sharded 4 images/core; weight+bias replicated.

Per-core Winograd F(2,3) along W (bf16 matmuls, fp32 PSUM):
  - W=56 output cols -> 28 tiles of 2.  For tile j the 4 input taps are
    padded cols 2j..2j+3.  Input transform (W only, per row):
      v0 = d0-d2, v1 = d1+d2, v2 = d2-d1, v3 = d1-d3
    computed on DVE (image 0, in 4 row-ranges right behind the DMA) or
    GPSIMD (images 1..3, one image ahead) into V[128, 4xi, 58, 28] bf16.
  - weights are host-transformed: U[xi][co,ci,kh] = sum_kw G[xi,kw] w[..kw],
    G = [[1,0,0],[.5,.5,.5],[.5,-.5,.5],[0,0,1]], laid out as
    [ci, ch, xi, kh, co128] bf16 so each (ch,xi,kh) is a [128,128] lhsT.
  - PE per (img, ch-half, 14-row chunk): 4 accumulation groups
    m_xi = sum_kh U[xi,kh]^T @ V[xi, 14r+kh : +14, :]   (12 matmuls of
    392 cols, vs 9x448 for direct conv = 1.5x fewer PE cycles).
    Group order m1,m2,m0,m3 so the combine can start mid-chunk.
  - combine (y0 = m0+m1+m2+b, y1 = m1-m2-m3+b) is spread over the other
    engines under the HW rule that a vector op reads at most one PSUM
    operand (and GPSIMD none):
      ACT:    e0 = Id(m0+b), e2 = Copy(m2), e3 = Copy(m3)
      DVE:    t01 = e0+m1, u12 = (m1+b)-e2, y0 = t01+e2 -> ot[..,0::2]
      GPSIMD: y1 = u12-e3 -> ot[..,1::2]
  - image 0 x is DMA'd in 4 disjoint row-ranges so the PE starts after
    ~230 KB lands; output of the last image DMAs out per row-chunk.
  - warm-up matmuls trip the PE HAM clock-gate during the startup window.
"""

import ml_dtypes
import numpy as np

import concourse.bacc as bacc
import concourse.mybir as mybir
import concourse.tile as tile
from concourse.bass_utils import run_bass_kernel_spmd

N_CORES = 8
N, C_IN, H, W = 32, 128, 56, 56
C_OUT, KH, KW = 256, 3, 3
PER = N // N_CORES          # images per core
HP, WP = H + 2, W + 2       # zero-padded image dims
NT = W // 2                 # 28 winograd tiles along W
NXI = 4                     # winograd terms
RPC = 14                    # output rows per chunk
N_CHUNKS = H // RPC         # 4
FD = RPC * NT               # 392 matmul cols per chunk
N_CH = C_OUT // 128         # output-channel halves
WARMUP_MMS = 3
# image-0 row ranges (disjoint, cover 0..57)
R0 = [(0, 16), (16, 30), (30, 44), (44, 58)]

f32 = mybir.dt.float32
bf16 = mybir.dt.bfloat16
ADD = mybir.AluOpType.add
SUB = mybir.AluOpType.subtract
COPY = mybir.ActivationFunctionType.Copy
IDENT = mybir.ActivationFunctionType.Identity

_prog_cache = {}


def _build_program():
    nc = bacc.Bacc("TRN2", target_bir_lowering=False, debug=False)
    # x is host-split into even/odd column planes [.., HP, 2, 29] and y is
    # produced parity-split [.., H, 2, 28] (host re-interleaves after the
    # run) so every vector-engine access on chip is contiguous.
    x_d = nc.declare_dram_parameter("x", [PER, C_IN, HP, 2, 29], bf16, isOutput=False)
    u_d = nc.declare_dram_parameter("u", [C_IN, N_CH, NXI, KH, 128], bf16, isOutput=False)
    b_d = nc.declare_dram_parameter("b", [128, N_CH], f32, isOutput=False)
    y_d = nc.declare_dram_parameter("y", [PER, N_CH, 128, 2, H, NT], f32, isOutput=True)

    with tile.TileContext(nc) as tc:
        with (
            tc.tile_pool(name="wpool", bufs=1) as wpool,
            tc.tile_pool(name="x0pool", bufs=4) as x0pool,
            tc.tile_pool(name="xppool", bufs=2) as xppool,
            tc.tile_pool(name="vpool", bufs=2) as vpool,
            tc.tile_pool(name="spool", bufs=4) as spool,
            tc.tile_pool(name="opool", bufs=3) as opool,
            tc.tile_pool(name="pspool", bufs=8, space="PSUM") as pspool,
            tc.tile_pool(name="warm", bufs=1) as warmpool,
        ):
            # PE warm-up during the startup protocol / first DMA window.
            wu_zero = warmpool.tile([128, FD], f32, tag="wuzero")
            nc.vector.memset(wu_zero[:], 0.0)
            wu_src = warmpool.tile([128, FD], bf16, tag="wusrc")
            nc.vector.tensor_copy(wu_src[:], wu_zero[:])
            wu_ps = pspool.tile([128, 512], f32, tag="ps")

            u_t = wpool.tile([C_IN, N_CH, NXI, KH, 128], bf16, tag="u")
            b_t = wpool.tile([128, N_CH], f32, tag="b")

            # image-0 row-range tiles + shared V tile
            x0c = [x0pool.tile([128, 16, 2, 29], bf16, tag="x0", name=f"x0c{k}")
                   for k in range(len(R0))]
            v0 = vpool.tile([128, NXI, HP, NT], bf16, tag="v")

            nc.sync.dma_start(u_t[:, 0], u_d[:, 0])
            nc.sync.dma_start(x0c[0][:, 0:R0[0][1] - R0[0][0]], x_d[0, :, R0[0][0]:R0[0][1]])

            for _ in range(WARMUP_MMS):
                nc.tensor.matmul(wu_ps[:, 0:FD], wu_src[:, :128], wu_src[:],
                                 start=True, stop=True)
            nc.scalar.dma_start(x0c[1][:, 0:R0[1][1] - R0[1][0]], x_d[0, :, R0[1][0]:R0[1][1]])
            nc.scalar.dma_start(b_t[:], b_d[:])

            def v_transform(eng, vt, xt, rows_out, rows_in):
                """vt[:, xi, rows_out, :] from xt rows rows_in (same rows).
                With d_t = xpad[2j+t]: v0=d0-d2, v1=d1+d2, v2=d2-d1,
                v3=d1-d3.  x is even/odd split: d0=xe[j], d2=xe[j+1],
                d1=xo[j], d3=xo[j+1] -- all contiguous 28-slices."""
                o0, o1 = rows_out
                i0, i1 = rows_in
                xe0 = xt[:, i0:i1, 0, 0:NT]
                xe1 = xt[:, i0:i1, 0, 1:NT + 1]
                xo0 = xt[:, i0:i1, 1, 0:NT]
                xo1 = xt[:, i0:i1, 1, 1:NT + 1]
                eng.tensor_tensor(vt[:, 0, o0:o1, :], xe0, xe1, SUB)
                eng.tensor_tensor(vt[:, 1, o0:o1, :], xo0, xe1, ADD)
                eng.tensor_tensor(vt[:, 2, o0:o1, :], xe1, xo0, SUB)
                eng.tensor_tensor(vt[:, 3, o0:o1, :], xo0, xo1, SUB)

            nc.sync.dma_start(x0c[2][:, 0:R0[2][1] - R0[2][0]],
                              x_d[0, :, R0[2][0]:R0[2][1]])
            nc.gpsimd.dma_start(x0c[3][:, 0:R0[3][1] - R0[3][0]],
                                x_d[0, :, R0[3][0]:R0[3][1]])
            nc.scalar.dma_start(u_t[:, 1], u_d[:, 1])

            # image-0 V on DVE right behind the DMAs, ahead of any combines
            for k in range(len(R0)):
                v_transform(nc.vector, v0, x0c[k], R0[k],
                            (0, R0[k][1] - R0[k][0]))

            vts = {0: v0}
            # matmul group order: combine can start after m1, m2
            XI_ORDER = (1, 2, 0, 3)

            for img in range(PER):
                vt = vts.pop(img)
                if img + 1 < PER:
                    xp = xppool.tile([128, HP, 2, 29], bf16, tag="xp")
                    nc.scalar.dma_start(xp[:], x_d[img + 1])
                    vnxt = vpool.tile([128, NXI, HP, NT], bf16, tag="v")
                    vts[img + 1] = vnxt
                for ch in range(N_CH):
                    ot = opool.tile([128, 2, H, NT], f32, tag="ot")
                    t01b = spool.tile([128, N_CHUNKS, RPC, NT], f32, tag="t01")
                    e2b = spool.tile([128, N_CHUNKS, RPC, NT], f32, tag="e2")
                    bias = b_t[:, ch:ch + 1]
                    for r in range(N_CHUNKS):
                        ms = {}
                        for xi in XI_ORDER:
                            ps = pspool.tile([128, 512], f32, tag="ps")
                            ms[xi] = ps
                            for kh in range(KH):
                                nc.tensor.matmul(
                                    ps[:, 0:FD],
                                    u_t[:, ch, xi, kh, :],
                                    vt[:, xi, RPC * r + kh:RPC * r + kh + RPC, :],
                                    start=(kh == 0),
                                    stop=(kh == KH - 1),
                                )
                        # combine: y0 = m0+m1+m2+b, y1 = m1-m2-m3+b.
                        # HW rule: each vector op may read at most ONE PSUM
                        # operand, and GPSIMD none, so ACT evicts m0/m2/m3
                        # while DVE pulls m1 straight from PSUM.
                        m0, m1, m2, m3 = (ms[i][:, 0:FD] for i in range(4))
                        e0 = spool.tile([128, RPC, NT], f32, tag="e0")
                        e3 = spool.tile([128, RPC, NT], f32, tag="e3")
                        u12 = spool.tile([128, RPC, NT], f32, tag="u12")
                        nc.scalar.activation(e2b[:, r], m2, COPY)
                        nc.vector.scalar_tensor_tensor(
                            u12[:], m1, bias, e2b[:, r], ADD, SUB)
                        nc.scalar.activation(e0[:], m0, IDENT, bias=bias)
                        nc.vector.tensor_tensor(t01b[:, r], e0[:], m1, ADD)
                        nc.scalar.activation(e3[:], m3, COPY)
                        rs = slice(RPC * r, RPC * (r + 1))
                        if img == PER - 1:
                            nc.vector.tensor_tensor(
                                ot[:, 0, rs, :], t01b[:, r], e2b[:, r], ADD)
                        nc.gpsimd.tensor_tensor(
                            ot[:, 1, rs, :], u12[:], e3[:], SUB)

                        if ch == 0 and r == 1 and img + 1 < PER:
                            # next image's V, whole image, on GPSIMD
                            v_transform(nc.gpsimd, vts[img + 1], xp,
                                        (0, HP), (0, HP))
                        if img == PER - 1:
                            nc.sync.dma_start(
                                y_d[img, ch, :, :, rs], ot[:, :, rs])
                    if img != PER - 1:
                        # whole even plane in one DVE op (amortizes the
                        # per-op read-write bubble), then one output DMA
                        nc.vector.tensor_tensor(
                            ot[:, 0], t01b[:], e2b[:], ADD)
                        nc.sync.dma_start(y_d[img, ch], ot[:])

    nc.compile()
    return nc


def _get_program():
    if "nc" not in _prog_cache:
        _prog_cache["nc"] = _build_program()
    return _prog_cache["nc"]


def _prep_inputs(x, weight, bias):
    x = np.ascontiguousarray(np.asarray(x, dtype=np.float32))
    weight = np.ascontiguousarray(np.asarray(weight, dtype=np.float32))
    bias = np.ascontiguousarray(np.asarray(bias, dtype=np.float32))

    x_pad = np.zeros((N, C_IN, HP, WP), dtype=ml_dtypes.bfloat16)
    x_pad[:, :, 1:1 + H, 1:1 + W] = x.astype(ml_dtypes.bfloat16)
    # even/odd column split: [n, ci, HP, 2, 29]
    x_pad = np.ascontiguousarray(
        x_pad.reshape(N, C_IN, HP, 29, 2).transpose(0, 1, 2, 4, 3))

    # U[xi][co,ci,kh] = sum_kw G[xi,kw] w[co,ci,kh,kw] -> [ci, ch, xi, kh, co128]
    G = np.array([[1, 0, 0], [.5, .5, .5], [.5, -.5, .5], [0, 0, 1]],
                 dtype=np.float32)
    u = np.einsum("gk,oihk->oihg", G, weight)      # [co, ci, kh, xi]
    u_t = np.ascontiguousarray(
        u.transpose(1, 2, 3, 0)                     # [ci, kh, xi, co]
        .reshape(C_IN, KH, NXI, N_CH, 128)
        .transpose(0, 3, 2, 1, 4)                   # [ci, ch, xi, kh, co128]
        .astype(ml_dtypes.bfloat16)
    )
    b_t = np.ascontiguousarray(bias.reshape(N_CH, 128).T)

    in_maps = []
    for c in range(N_CORES):
        in_maps.append({
            "x": x_pad[c * PER:(c + 1) * PER],
            "u": u_t,
            "b": b_t,
        })
    return in_maps


def _run(x, weight, bias, trace=False):
    nc = _get_program()
    in_maps = _prep_inputs(x, weight, bias)
    res = run_bass_kernel_spmd(
        nc, in_maps, core_ids=list(range(N_CORES)), trace=trace,
    )
    # y arrives parity-split [PER, N_CH, 128, 2, H, NT]; re-interleave W.
    parts = [
        res.results[c]["y"].reshape(PER, C_OUT, 2, H, NT)
        .transpose(0, 1, 3, 4, 2).reshape(PER, C_OUT, H, W)
        for c in range(N_CORES)
    ]
    y = np.concatenate(parts, axis=0)
    return y, res


def kernel(x, weight, bias):
    y, _ = _run(x, weight, bias, trace=False)
    return y


# revision 19
# speedup vs baseline: 1.0226x; 1.0226x over previous
"""Conv2d 3x3 same-padding, NCHW, on 8 TRN2 NeuronCores (data-parallel).

Problem: x[32,128,56,56] f32, weight[256,128,3,3] OIHW, bias[256] ->
y[32,256,56,56].  Batch is sharded 4 images/core; weight+bias replicated.

Per-core Winograd F(2,3) along W (bf16 matmuls, fp32 PSUM):
  - W=56 output cols -> 28 tiles of 2.  For tile j the 4 input taps are
    padded cols 2j..2j+3.  Input transform (W only, per row):
      v0 = d0-d2, v1 = d1+d2, v2 = d2-d1, v3 = d1-d3
    computed on DVE (image 0, in 4 row-ranges right behind the DMA) or
    GPSIMD (images 1..3, one image ahead) into V[128, 4xi, 58, 28] bf16.
  - weights are host-transformed: U[xi][co,ci,kh] = sum_kw G[xi,kw] w[..kw],
    G = [[1,0,0],[.5,.5,.5],[.5,-.5,.5],[0,0,1]], laid out as
    [ci, ch, xi, kh, co128] bf16 so each (ch,xi,kh) is a [128,128] lhsT.
  - PE per (img, ch-half, 14-row chunk): 4 accumulation groups
    m_xi = sum_kh U[xi,kh]^T @ V[xi, 14r+kh : +14, :]   (12 matmuls of
    392 cols, vs 9x448 for direct conv = 1.5x fewer PE cycles).
    Group order m1,m2,m0,m3 so the combine can start mid-chunk.
  - combine (y0 = m0+m1+m2+b, y1 = m1-m2-m3+b) is spread over the other
    engines under the HW rule that a vector op reads at most one PSUM
    operand (and GPSIMD none):
      ACT:    e0 = Id(m0+b), e2 = Copy(m2), e3 = Copy(m3)
      DVE:    t01 = e0+m1, u12 = (m1+b)-e2, y0 = t01+e2 -> ot[..,0::2]
      GPSIMD: y1 = u12-e3 -> ot[..,1::2]
  - image 0 x is DMA'd in 4 disjoint row-ranges so the PE starts after
    ~230 KB lands; output of the last image DMAs out per row-chunk.
  - warm-up matmuls trip the PE HAM clock-gate during the startup window.
"""

import ml_dtypes
import numpy as np

import concourse.bacc as bacc
import concourse.mybir as mybir
import concourse.tile as tile
from concourse.bass_utils import run_bass_kernel_spmd

N_CORES = 8
N, C_IN, H, W = 32, 128, 56, 56
C_OUT, KH, KW = 256, 3, 3
PER = N // N_CORES          # images per core
HP, WP = H + 2, W + 2       # zero-padded image dims
NT = W // 2                 # 28 winograd tiles along W
NXI = 4                     # winograd terms
RPC = 14                    # output rows per chunk
N_CHUNKS = H // RPC         # 4
FD = RPC * NT               # 392 matmul cols per chunk
N_CH = C_OUT // 128         # output-channel halves
WARMUP_MMS = 3
# image-0 row ranges (disjoint, cover 0..57)
R0 = [(0, 16), (16, 30), (30, 44), (44, 58)]

f32 = mybir.dt.float32
bf16 = mybir.dt.bfloat16
ADD = mybir.AluOpType.add
SUB = mybir.AluOpType.subtract
COPY = mybir.ActivationFunctionType.Copy
IDENT = mybir.ActivationFunctionType.Identity

_prog_cache = {}


def _build_program():
    nc = bacc.Bacc("TRN2", target_bir_lowering=False, debug=False)
    # x is host-split into even/odd column planes [.., HP, 2, 29] and y is
    # produced parity-split [.., H, 2, 28] (host re-interleaves after the
    # run) so every vector-engine access on chip is contiguous.
    x_d = nc.declare_dram_parameter("x", [PER, C_IN, HP, 2, 29], bf16, isOutput=False)
    u_d = nc.declare_dram_parameter("u", [C_IN, N_CH, NXI, KH, 128], bf16, isOutput=False)
    b_d = nc.declare_dram_parameter("b", [128, N_CH], f32, isOutput=False)
    y_d = nc.declare_dram_parameter("y", [PER, N_CH, 128, 2, H, NT], f32, isOutput=True)

    with tile.TileContext(nc) as tc:
        with (
            tc.tile_pool(name="wpool", bufs=1) as wpool,
            tc.tile_pool(name="x0pool", bufs=4) as x0pool,
            tc.tile_pool(name="xppool", bufs=2) as xppool,
            tc.tile_pool(name="vpool", bufs=2) as vpool,
            tc.tile_pool(name="spool", bufs=4) as spool,
            tc.tile_pool(name="opool", bufs=3) as opool,
            tc.tile_pool(name="pspool", bufs=8, space="PSUM") as pspool,
            tc.tile_pool(name="warm", bufs=1) as warmpool,
        ):
            # PE warm-up during the startup protocol / first DMA window.
            wu_zero = warmpool.tile([128, FD], f32, tag="wuzero")
            nc.vector.memset(wu_zero[:], 0.0)
            wu_src = warmpool.tile([128, FD], bf16, tag="wusrc")
            nc.vector.tensor_copy(wu_src[:], wu_zero[:])
            wu_ps = pspool.tile([128, 512], f32, tag="ps")

            u_t = wpool.tile([C_IN, N_CH, NXI, KH, 128], bf16, tag="u")
            b_t = wpool.tile([128, N_CH], f32, tag="b")

            # image-0 row-range tiles + shared V tile
            x0c = [x0pool.tile([128, 16, 2, 29], bf16, tag="x0", name=f"x0c{k}")
                   for k in range(len(R0))]
            v0 = vpool.tile([128, NXI, HP, NT], bf16, tag="v")

            nc.sync.dma_start(u_t[:, 0], u_d[:, 0])
            nc.sync.dma_start(x0c[0][:, 0:R0[0][1] - R0[0][0]], x_d[0, :, R0[0][0]:R0[0][1]])

            for _ in range(WARMUP_MMS):
                nc.tensor.matmul(wu_ps[:, 0:FD], wu_src[:, :128], wu_src[:],
                                 start=True, stop=True)
            nc.scalar.dma_start(x0c[1][:, 0:R0[1][1] - R0[1][0]], x_d[0, :, R0[1][0]:R0[1][1]])
            nc.scalar.dma_start(b_t[:], b_d[:])

            def v_transform(eng, vt, xt, rows_out, rows_in):
                """vt[:, xi, rows_out, :] from xt rows rows_in (same rows).
                With d_t = xpad[2j+t]: v0=d0-d2, v1=d1+d2, v2=d2-d1,
                v3=d1-d3.  x is even/odd split: d0=xe[j], d2=xe[j+1],
                d1=xo[j], d3=xo[j+1] -- all contiguous 28-slices."""
                o0, o1 = rows_out
                i0, i1 = rows_in
                xe0 = xt[:, i0:i1, 0, 0:NT]
                xe1 = xt[:, i0:i1, 0, 1:NT + 1]
                xo0 = xt[:, i0:i1, 1, 0:NT]
                xo1 = xt[:, i0:i1, 1, 1:NT + 1]
                eng.tensor_tensor(vt[:, 0, o0:o1, :], xe0, xe1, SUB)
                eng.tensor_tensor(vt[:, 1, o0:o1, :], xo0, xe1, ADD)
                eng.tensor_tensor(vt[:, 2, o0:o1, :], xe1, xo0, SUB)
                eng.tensor_tensor(vt[:, 3, o0:o1, :], xo0, xo1, SUB)

            nc.sync.dma_start(x0c[2][:, 0:R0[2][1] - R0[2][0]],
                              x_d[0, :, R0[2][0]:R0[2][1]])
            nc.gpsimd.dma_start(x0c[3][:, 0:R0[3][1] - R0[3][0]],
                                x_d[0, :, R0[3][0]:R0[3][1]])
            nc.scalar.dma_start(u_t[:, 1], u_d[:, 1])

            # image-0 V on DVE right behind the DMAs, ahead of any combines
            for k in range(len(R0)):
                v_transform(nc.vector, v0, x0c[k], R0[k],
                            (0, R0[k][1] - R0[k][0]))

            vts = {0: v0}
            # matmul group order: combine can start after m1, m2
            XI_ORDER = (1, 2, 0, 3)

            for img in range(PER):
                vt = vts.pop(img)
                if img + 1 < PER:
                    xp = xppool.tile([128, HP, 2, 29], bf16, tag="xp")
                    nc.scalar.dma_start(xp[:], x_d[img + 1])
                    vnxt = vpool.tile([128, NXI, HP, NT], bf16, tag="v")
                    vts[img + 1] = vnxt
                for ch in range(N_CH):
                    ot = opool.tile([128, 2, H, NT], f32, tag="ot")
                    t01b = spool.tile([128, N_CHUNKS, RPC, NT], f32, tag="t01")
                    e2b = spool.tile([128, N_CHUNKS, RPC, NT], f32, tag="e2")
                    u12b = spool.tile([128, N_CHUNKS, RPC, NT], f32, tag="u12")
                    e3b = spool.tile([128, N_CHUNKS, RPC, NT], f32, tag="e3")
                    bias = b_t[:, ch:ch + 1]
                    for r in range(N_CHUNKS):
                        ms = {}
                        for xi in XI_ORDER:
                            ps = pspool.tile([128, 512], f32, tag="ps")
                            ms[xi] = ps
                            for kh in range(KH):
                                nc.tensor.matmul(
                                    ps[:, 0:FD],
                                    u_t[:, ch, xi, kh, :],
                                    vt[:, xi, RPC * r + kh:RPC * r + kh + RPC, :],
                                    start=(kh == 0),
                                    stop=(kh == KH - 1),
                                )
                        # combine: y0 = m0+m1+m2+b, y1 = m1-m2-m3+b.
                        # HW rule: each vector op may read at most ONE PSUM
                        # operand, and GPSIMD none, so ACT evicts m0/m2/m3
                        # while DVE pulls m1 straight from PSUM.
                        m0, m1, m2, m3 = (ms[i][:, 0:FD] for i in range(4))
                        e0 = spool.tile([128, RPC, NT], f32, tag="e0")
                        nc.scalar.activation(e2b[:, r], m2, COPY)
                        nc.vector.scalar_tensor_tensor(
                            u12b[:, r], m1, bias, e2b[:, r], ADD, SUB)
                        nc.scalar.activation(e0[:], m0, IDENT, bias=bias)
                        nc.vector.tensor_tensor(t01b[:, r], e0[:], m1, ADD)
                        nc.scalar.activation(e3b[:, r], m3, COPY)
                        rs = slice(RPC * r, RPC * (r + 1))
                        if img == PER - 1:
                            nc.vector.tensor_tensor(
                                ot[:, 0, rs, :], t01b[:, r], e2b[:, r], ADD)
                            nc.gpsimd.tensor_tensor(
                                ot[:, 1, rs, :], u12b[:, r], e3b[:, r], SUB)

                        if ch == 0 and r == 1 and img + 1 < PER:
                            # next image's V, whole image, on GPSIMD
                            v_transform(nc.gpsimd, vts[img + 1], xp,
                                        (0, HP), (0, HP))
                        if img == PER - 1:
                            nc.sync.dma_start(
                                y_d[img, ch, :, :, rs], ot[:, :, rs])
                    if img != PER - 1:
                        # whole planes in one DVE/GPSIMD op each (amortizes
                        # the per-op launch overhead), then one output DMA
                        nc.vector.tensor_tensor(
                            ot[:, 0], t01b[:], e2b[:], ADD)
                        nc.gpsimd.tensor_tensor(
                            ot[:, 1], u12b[:], e3b[:], SUB)
                        nc.sync.dma_start(y_d[img, ch], ot[:])

    nc.compile()
    return nc


def _get_program():
    if "nc" not in _prog_cache:
        _prog_cache["nc"] = _build_program()
    return _prog_cache["nc"]


def _prep_inputs(x, weight, bias):
    x = np.ascontiguousarray(np.asarray(x, dtype=np.float32))
    weight = np.ascontiguousarray(np.asarray(weight, dtype=np.float32))
    bias = np.ascontiguousarray(np.asarray(bias, dtype=np.float32))

    x_pad = np.zeros((N, C_IN, HP, WP), dtype=ml_dtypes.bfloat16)
    x_pad[:, :, 1:1 + H, 1:1 + W] = x.astype(ml_dtypes.bfloat16)
    # even/odd column split: [n, ci, HP, 2, 29]
    x_pad = np.ascontiguousarray(
        x_pad.reshape(N, C_IN, HP, 29, 2).transpose(0, 1, 2, 4, 3))

    # U[xi][co,ci,kh] = sum_kw G[xi,kw] w[co,ci,kh,kw] -> [ci, ch, xi, kh, co128]
    G = np.array([[1, 0, 0], [.5, .5, .5], [.5, -.5, .5], [0, 0, 1]],
                 dtype=np.float32)
    u = np.einsum("gk,oihk->oihg", G, weight)      # [co, ci, kh, xi]
    u_t = np.ascontiguousarray(
        u.transpose(1, 2, 3, 0)                     # [ci, kh, xi, co]
        .reshape(C_IN, KH, NXI, N_CH, 128)
        .transpose(0, 3, 2, 1, 4)                   # [ci, ch, xi, kh, co128]
        .astype(ml_dtypes.bfloat16)
    )
    b_t = np.ascontiguousarray(bias.reshape(N_CH, 128).T)

    in_maps = []
    for c in range(N_CORES):
        in_maps.append({
            "x": x_pad[c * PER:(c + 1) * PER],
            "u": u_t,
            "b": b_t,
        })
    return in_maps


def _run(x, weight, bias, trace=False):
    nc = _get_program()
    in_maps = _prep_inputs(x, weight, bias)
    res = run_bass_kernel_spmd(
        nc, in_maps, core_ids=list(range(N_CORES)), trace=trace,
    )
    # y arrives parity-split [PER, N_CH, 128, 2, H, NT]; re-interleave W.
    parts = [
        res.results[c]["y"].reshape(PER, C_OUT, 2, H, NT)
        .transpose(0, 1, 3, 4, 2).reshape(PER, C_OUT, H, W)
        for c in range(N_CORES)
    ]
    y = np.concatenate(parts, axis=0)
    return y, res


def kernel(x, weight, bias):
    y, _ = _run(x, weight, bias, trace=False)
    return y
